# revision 1
# baseline (speedup 1.0000x reference)
"""GPT2 causal self-attention on 8 TRN2 NeuronCores.

Sharding: tensor-parallel over heads (4 heads/core) x data-parallel over
batch (B=2 -> 2 groups of 4 cores). Each core computes, for its batch
element b and head group g (heads 4g..4g+3):
    qkv slice -> causal attention over its 4 heads -> partial c_proj
and returns the partial [C, T] projection output (transposed, bf16). The
host sums the 4 partials per batch element (the TP all-reduce); bproj is
applied on the g==0 core only.

Design (cost-model engine budget: PE ~89us after fp8-S, Act ~97us):
- QKV: x^T [128,8,T] and Wqkv [128,8,768] arrive in a few large DMAs
  (HWDGE issue is ~625ns serialized, so DMA count matters) split across
  the SP and Activation queues in consumption order.
- Q^T/K^T are written as fp8e4m3 (+bias via the DVE copy) and repacked
  by SBUF->SBUF DMA into a DoubleRow pair layout [32*(h%2)+p, i, t]
  (head-dim d = 32i + p, two heads per tensor at base partitions 0/32),
  so each S^T chunk is a single half-cost DoubleRow fp8 matmul.
- exp runs on Activation: two consecutive same-head S chunks share one
  2-bank psum tile and ONE exp instruction (halves the ~185ns per-exp
  access overhead; chunk b computes its full 512 cols so the shared exp
  never reads uninitialized psum), from a 2-deep pair ring = 4-chunk
  lookahead (depth beats wider tiles: narrow diagonal exps otherwise
  bubble on the S->exp->S feedback). tri-mask on Pool (GPSIMD cannot
  access PSUM on real HW - only sbuf->sbuf work there). y-scale,
  V/yT/proj copies on DVE.
- Emission is driven by a greedy clock model: S/exp chunks paced
  against a modeled Activation clock with a LEAD margin, filler PE work
  (QKV blocks 2-3, V tiles, y groups, transposes, proj groups) popped
  from a dependency-tracked list in the slack slots. Heads are paired,
  chunks head-major within the pair, so transposes/proj unlock early.

Layouts on device (feature-major to avoid transposes):
  xt   = x[b].T            [C=1024, T=2048] bf16 as one [128,8,2048] tile
  Q^T, K^T                 fp8 [128, T] + DoubleRow-paired [128, 2, T]
  V                        token-major [T, 256] as 16x[128, 4, 65] tiles
                           (65th column per head = 1.0 for the softmax denom)
  S^T pair tile            2x[kt=128, qt<=512] psum (2 banks), exp -> P^T bf16
  y accum                  [qt=128, 65] psum (col 64 = denominator)
  y, y^T, out^T            via PE transpose + proj matmul
"""

import numpy as np
import ml_dtypes

import concourse.bass as bass
import concourse.bacc as bacc
import concourse.mybir as mybir
from concourse.tile import TileContext
from concourse.bass_utils import run_bass_kernel_spmd
from concourse.masks import make_identity

B, T, C = 2, 2048, 1024
H, HD = 16, 64
NCORES = 8
HPG = 4            # heads per core
DG = HPG * HD      # 256 feature dims per core
import os as _os
TB = 512           # qkv/proj column block
TBQ = int(_os.environ.get("KTBQ", "512"))   # attention query block width
NJ = T // TBQ      # attention query blocks
CPB = TBQ // 128   # key chunks per query block increment / qt-subs per block
SBUFS = 2   # S pair-tile ring depth (2-bank tiles, 4 banks total)
LEAD = float(_os.environ.get("KLEAD", "3900"))
PWB = int(_os.environ.get("KPW", "52"))
NTT = T // 128     # 16 token tiles

BF16 = mybir.dt.bfloat16
F32 = mybir.dt.float32
FP8 = mybir.dt.float8e4
AF = mybir.ActivationFunctionType

_BF16_NP = ml_dtypes.bfloat16


def _build():
    nc = bacc.Bacc()
    xt_d = nc.declare_dram_parameter("xt", [C, T], BF16, isOutput=False)
    wqkv_d = nc.declare_dram_parameter("wqkv", [C, 3 * DG], BF16, isOutput=False)
    bqk_d = nc.declare_dram_parameter("bqk", [128, 4], F32, isOutput=False)
    bv_d = nc.declare_dram_parameter("bv", [DG], BF16, isOutput=False)
    wp_d = nc.declare_dram_parameter("wp", [DG, C], BF16, isOutput=False)
    bp_d = nc.declare_dram_parameter("bp", [128, 8], F32, isOutput=False)
    tri_d = nc.declare_dram_parameter("tri", [128, 128], BF16, isOutput=False)
    out_d = nc.declare_dram_parameter("out", [C, T], BF16, isOutput=True)

    xt_v = xt_d.rearrange("(kc p) t -> p kc t", p=128)     # [128, 8, T]
    wq_v = wqkv_d.rearrange("(kc p) c -> p kc c", p=128)   # [128, 8, 768]
    wp_v = wp_d.rearrange("(g p) c -> p g c", p=128)       # [128, 2, 1024]

    with TileContext(nc) as tc:
        with (
            tc.tile_pool(name="const", bufs=1) as cp,
            tc.tile_pool(name="pwork", bufs=PWB) as pw,
            tc.tile_pool(name="ostage", bufs=4) as op,
            tc.tile_pool(name="recips", bufs=8) as rp,
            tc.tile_pool(name="psmm", bufs=2, space="PSUM") as psmm,
            tc.tile_pool(name="pss", bufs=SBUFS, space="PSUM") as pss,
            tc.tile_pool(name="psy", bufs=2, space="PSUM") as psy,
        ):
            # ---- persistent sbuf tensors ----
            xt = cp.tile([128, 8, T], BF16, tag="xt")
            wq = cp.tile([128, 8, 3 * DG], BF16, tag="wq")
            wp = cp.tile([128, 2, C], BF16, tag="wp")
            q8c = [cp.tile([128, T], FP8, tag=f"q8c{g}", name=f"q8c{g}") for g in range(2)]
            k8c = [cp.tile([128, T], FP8, tag=f"k8c{g}", name=f"k8c{g}") for g in range(2)]
            # head-paired fp8 layout for DoubleRow S matmuls: two heads per
            # tensor at base partitions 0/32 (PE requires base 0/32/64);
            # partition 32*(h%2)+p, pair index i <-> head-dim d = 32i + p
            q8p = [cp.tile([128, 2, T], FP8, tag=f"q8p{a}", name=f"q8p{a}") for a in range(2)]
            k8p = [cp.tile([128, 2, T], FP8, tag=f"k8p{a}", name=f"k8p{a}") for a in range(2)]
            vsb = [cp.tile([128, HPG, HD + 1], BF16, tag=f"v{t}", name=f"v{t}") for t in range(NTT)]
            ysb = [cp.tile([128, DG], BF16, tag=f"y{t}", name=f"y{t}") for t in range(NTT)]
            yT = [cp.tile([128, T], BF16, tag=f"yT{g}", name=f"yT{g}") for g in range(2)]
            tri = cp.tile([128, 128], BF16, tag="tri")
            ident = cp.tile([128, 128], BF16, tag="ident")
            bqk = cp.tile([128, 4], F32, tag="bqk")
            bp = cp.tile([128, 8], F32, tag="bp")
            bv = cp.tile([1, DG], BF16, tag="bv")
            ones = cp.tile([1, 128], BF16, tag="ones")

            # ---- input DMAs: few large transfers, split across the SP and
            # Activation HWDGE queues in consumption order.
            nc.sync.dma_start(out=xt[:, :, 0:TB], in_=xt_v[:, :, 0:TB])
            nc.scalar.dma_start(out=wq[:, :, 0:DG], in_=wq_v[:, :, 0:DG])
            nc.scalar.dma_start(out=wq[:, :, DG:2 * DG], in_=wq_v[:, :, DG:2 * DG])
            nc.scalar.dma_start(out=bqk, in_=bqk_d[:, :])
            nc.scalar.dma_start(out=bv, in_=bv_d[:].rearrange("(o n) -> o n", o=1))
            nc.scalar.dma_start(out=tri, in_=tri_d[:, :])
            nc.sync.dma_start(out=xt[:, :, TB:2 * TB], in_=xt_v[:, :, TB:2 * TB])
            nc.sync.dma_start(out=wq[:, :, 2 * DG:3 * DG], in_=wq_v[:, :, 2 * DG:3 * DG])
            nc.sync.dma_start(out=xt[:, :, 2 * TB:3 * TB], in_=xt_v[:, :, 2 * TB:3 * TB])
            nc.sync.dma_start(out=xt[:, :, 3 * TB:4 * TB], in_=xt_v[:, :, 3 * TB:4 * TB])
            nc.sync.dma_start(out=wp, in_=wp_v)
            nc.sync.dma_start(out=bp, in_=bp_d[:, :])
            make_identity(nc, ident[:, :])
            nc.vector.memset(ones[:, :], 1.0)
            for t in range(NTT):
                nc.vector.memset(vsb[t][:, :, HD:HD + 1], 1.0)

            def qk_group(n, qk, g):
                # Q^T / K^T feature-major: out[dim, tok] = W.T @ x.T.
                # Returns two closures (half-groups) sized for slack slots.
                dst = (q8c if qk == 0 else k8c)[g]
                col = qk * DG + g * 128
                box = {}

                def first():
                    ps = psmm.tile([128, TB], F32, tag="mm", name="ps")
                    box["ps"] = ps
                    for kc in range(4):
                        nc.tensor.matmul(
                            ps,
                            lhsT=wq[:, kc, col:col + 128],
                            rhs=xt[:, kc, n * TB:(n + 1) * TB],
                            start=(kc == 0), stop=False,
                        )

                def second():
                    ps = box["ps"]
                    for kc in range(4, 8):
                        nc.tensor.matmul(
                            ps,
                            lhsT=wq[:, kc, col:col + 128],
                            rhs=xt[:, kc, n * TB:(n + 1) * TB],
                            start=False, stop=(kc == 7),
                        )
                    if n < 2:
                        # head region: Act is idle before its first exp, so
                        # these copies ride there and unload DVE
                        nc.scalar.activation(
                            out=dst[:, n * TB:(n + 1) * TB], in_=ps,
                            func=AF.Identity,
                            bias=bqk[:, 2 * qk + g:2 * qk + g + 1],
                        )
                    else:
                        nc.vector.tensor_scalar_add(
                            dst[:, n * TB:(n + 1) * TB],
                            ps,
                            bqk[:, 2 * qk + g:2 * qk + g + 1],
                        )

                return [first, second]

            def v_tile(t):
                # V token-major: out[tok, vdim] = x @ Wv  (+ bv via rank-1)
                ps = psmm.tile([128, DG], F32, tag="mm", name="ps")
                for kc in range(8):
                    nc.tensor.matmul(
                        ps,
                        lhsT=xt[:, kc, t * 128:(t + 1) * 128],
                        rhs=wq[:, kc, 2 * DG:3 * DG],
                        start=(kc == 0), stop=False,
                    )
                nc.tensor.matmul(ps, lhsT=ones, rhs=bv, start=False, stop=True)
                nc.vector.tensor_copy(
                    out=vsb[t][:, :, 0:HD],
                    in_=ps.rearrange("p (h d) -> p h d", h=HPG),
                )

            def emit_transpose(t):
                for g in range(2):
                    tp = psy.tile([128, 128], BF16, tag="y", name="tp")
                    nc.tensor.transpose(
                        out=tp, in_=ysb[t][:, g * 128:(g + 1) * 128],
                        identity=ident,
                    )
                    nc.vector.tensor_copy(
                        out=yT[g][:, t * 128:(t + 1) * 128], in_=tp,
                    )

            def proj_group(n, cc, ost=None, c_lo=0, c_hi=TB, dma=True):
                # c_proj: out^T[c, tok] = Wp.T @ y^T for token block n (512),
                # optionally a col sub-range accumulated into a caller tile.
                # The last block alternates psum pools (psy is idle by then)
                # so the ring doesn't serialize the tail.
                ps = psmm.tile([128, c_hi - c_lo], F32, tag="mm", name="ps")
                for g in range(2):
                    nc.tensor.matmul(
                        ps,
                        lhsT=wp[:, g, cc * 128:(cc + 1) * 128],
                        rhs=yT[g][:, n * TB + c_lo:n * TB + c_hi],
                        start=(g == 0), stop=(g == 1),
                    )
                if ost is None:
                    ost = op.tile([128, TB], BF16, tag="o", name="ost")
                if n == T // TB - 1 and cc % 2 == 0:
                    # Act is idle after its last exp; halve the copy tail
                    nc.scalar.activation(
                        out=ost[:, c_lo:c_hi], in_=ps,
                        func=AF.Identity, bias=bp[:, cc:cc + 1],
                    )
                else:
                    nc.vector.tensor_scalar_add(
                        ost[:, c_lo:c_hi], ps, bp[:, cc:cc + 1])
                if dma:
                    eng = nc.scalar if (n == T // TB - 1 and cc % 2 == 0) else nc.sync
                    eng.dma_start(
                        out=out_d[cc * 128:(cc + 1) * 128, n * TB:(n + 1) * TB],
                        in_=ost,
                    )

            pts_live = {(n, h): [] for n in range(NJ) for h in range(HPG)}

            def emit_y(n, h, i, post_i=None):
                # One y accumulation group per qt-sub i: start=True clears
                # has_written for the whole psum bank, so groups must not
                # interleave within a bank.
                pts = pts_live[(n, h)]
                t = CPB * n + i
                last = CPB * n + i
                yps = psy.tile([128, HD + 1], F32, tag="y", name="yps")
                for kk in range(last + 1):
                    off = TBQ * (kk % 2) + i * 128
                    nc.tensor.matmul(
                        yps,
                        lhsT=pts[kk // 2][:, off:off + 128],
                        rhs=vsb[kk][:, h, :],
                        start=(kk == 0), stop=(kk == last),
                    )
                rec = rp.tile([128, 1], F32, tag="r", name="rec")
                nc.vector.reciprocal(out=rec, in_=yps[:, HD:HD + 1])
                nc.vector.tensor_scalar_mul(
                    ysb[t][:, h * HD:(h + 1) * HD],
                    yps[:, 0:HD],
                    rec,
                )
                if post_i is not None:
                    post_i(i, t)

            def emit_s_pair(n, h, r):
                # Two consecutive S^T chunks (keys 256r..256r+256) share one
                # 2-bank psum tile and ONE exp instruction, halving the
                # per-exp access overhead. Chunk b computes its full 512
                # columns (cheap fp8 matmul) so the shared exp never reads
                # uninitialized psum. DoubleRow fp8: contraction d = 32i + p.
                ka, kb = 2 * r, 2 * r + 1
                c0a = max(128 * ka - TBQ * n, 0)
                c0b = max(128 * kb - TBQ * n, 0)
                sp = pss.tile([128, 2 * TBQ], F32, tag="s", name="sp")
                hb = 32 * (h % 2)
                for k, lo in ((ka, c0a), (kb, TBQ)):
                    nc.tensor.matmul(
                        sp[:, lo:(TBQ if k == ka else 2 * TBQ)],
                        lhsT=k8p[h // 2][hb:hb + 32, :, k * 128:(k + 1) * 128],
                        rhs=q8p[h // 2][hb:hb + 32, :,
                                        TBQ * n + (lo - (0 if k == ka else TBQ)):TBQ * (n + 1)],
                        start=True, stop=True,
                        perf_mode=mybir.MatmulPerfMode.DoubleRow,
                    )
                pt = pw.tile([128, 2 * TBQ], BF16, tag="p", name=f"pt{r}")
                nc.scalar.activation(
                    out=pt[:, c0a:2 * TBQ], in_=sp[:, c0a:2 * TBQ],
                    func=AF.Exp, scale=0.125,
                )
                for k, base, c0 in ((ka, 0, c0a), (kb, TBQ, c0b)):
                    if 128 * k >= TBQ * n:
                        nc.gpsimd.tensor_mul(
                            out=pt[:, base + c0:base + c0 + 128],
                            in0=pt[:, base + c0:base + c0 + 128],
                            in1=tri,
                        )
                pts_live[(n, h)].append(pt)

            # ---- greedy interleaved emission ----
            # Pace S/exp chunk emission against a clock model of the
            # Activation engine: after each chunk, emit filler PE work until
            # the modeled PE clock catches the Act clock. Heads are paired
            # and chunk emission alternates within a pair (k-major) so yT
            # transposes and proj unlock early.
            import os
            expf = float(os.environ.get("KEXPF", "1.05"))

            def expc(n, r):
                c0a = max(256 * r - TBQ * n, 0)
                return ((2 * TBQ - c0a) * 0.833 + 185) * expf

            def scc(n, r):
                c0a = max(256 * r - TBQ * n, 0)
                return (2 * TBQ - c0a) * 0.104 + 120

            chunks_done = {(n, h): 0 for n in range(NJ) for h in range(HPG)}
            y_done = {(n, h): 0 for n in range(NJ) for h in range(HPG)}
            state = {"v": 0, "T": 0}

            fillers = []        # (cost_ns, ready_fn, emit_fn, emitted)

            def add_filler(cost, ready, emit):
                fillers.append([cost, ready, emit, False])

            # qkv column blocks 2,3 (needed before any n>=2 chunk)
            qk23 = {0: [], 1: []}
            for g in range(2):
                for qk in range(2):
                    for nn in (2, 3):
                        qk23[g].append(qk_group(nn, qk, g))
            shuffled1 = {0: False, 1: False}

            def emit_shuffle1_g(g):
                lo, hi = T // 2, T
                for hh in (2 * g, 2 * g + 1):
                    po = (hh % 2) * 64
                    hb = 32 * (hh % 2)
                    for src_, dst in ((q8c, q8p), (k8c, k8p)):
                        for i in range(2):
                            nc.sync.dma_start(
                                out=dst[g][hb:hb + 32, i, lo:hi],
                                in_=src_[g][po + 32 * i:po + 32 * i + 32, lo:hi],
                            )

            def force_qk23(g):
                if shuffled1[g]:
                    return
                for f in fillers:
                    if not f[3] and getattr(f[2], "__name__", "") == f"qk23_{g}":
                        f[2]()
                        f[3] = True
                emit_shuffle1_g(g)
                shuffled1[g] = True

            qk23_left = {0: 4, 1: 4}
            for g in range(2):
                for pieces in qk23[g]:
                    def mk(pieces=pieces, g=g):
                        def emitfn():
                            for p in pieces:
                                p()
                            qk23_left[g] -= 1
                            if qk23_left[g] == 0 and not shuffled1[g]:
                                emit_shuffle1_g(g)
                                shuffled1[g] = True
                        emitfn.__name__ = f"qk23_{g}"
                        return emitfn
                    add_filler(1700, lambda: True, mk())

            for t in range(NTT):
                def mkv(t=t):
                    def emitfn():
                        v_tile(t)
                        state["v"] += 1
                    return emitfn
                add_filler(960, lambda: True, mkv())

            for n in range(NJ):
                for i in range(CPB):
                    t = CPB * n + i
                    for h in range(HPG):
                        def mky(n=n, h=h, i=i):
                            def emitfn():
                                emit_y(n, h, i)
                                y_done[(n, h)] += 1
                            return emitfn
                        add_filler(
                            27 * (CPB * n + i + 1) + 120,
                            lambda n=n, h=h, i=i: (
                                chunks_done[(n, h)] > CPB * n + i
                                and state["v"] > CPB * n + i
                            ),
                            mky(),
                        )
                    def mkt(t=t, n=n, i=i):
                        def emitfn():
                            emit_transpose(t)
                            state["T"] += 1
                        return emitfn
                    add_filler(
                        160,
                        lambda n=n, i=i: all(
                            y_done[(n, h)] > i for h in range(HPG)
                        ),
                        mkt(),
                    )
                    if t % 4 == 3:
                        for cc in range(8):
                            def mkp(nb=t // 4, cc=cc):
                                def emitfn():
                                    proj_group(nb, cc)
                                return emitfn
                            add_filler(
                                426,
                                lambda t=t: state["T"] >= t + 1,
                                mkp(),
                            )

            def pop_filler():
                for f in fillers:
                    if not f[3] and f[1]():
                        f[2]()
                        f[3] = True
                        return f[0]
                return None

            # head: g-major QKV for the first two column blocks, then the
            # j<2 shuffle, so the first head pair's chunks unlock after 4
            # matmul groups
            clock = {"pe": 0.0, "act": 0.0}
            for g in range(2):
                for qk in range(2):
                    for nn in (0, 1):
                        for piece in qk_group(nn, qk, g):
                            piece()
                lo, hi = 0, T // 2
                for hh in (2 * g, 2 * g + 1):
                    po = (hh % 2) * 64
                    hb = 32 * (hh % 2)
                    for src_, dst in ((q8c, q8p), (k8c, k8p)):
                        for i in range(2):
                            nc.sync.dma_start(
                                out=dst[g][hb:hb + 32, i, lo:hi],
                                in_=src_[g][po + 32 * i:po + 32 * i + 32, lo:hi],
                            )
                clock["pe"] += 4 * 1700

            import os as _os2
            _seqv = _os2.environ.get("KSEQ", "0")
            chunk_seq = []           # units are chunk PAIRS (r = k//2)
            if _seqv == "1":
                for n in range(NJ):
                    for r in range(CPB * (n + 1) // 2):
                        for h in range(HPG):
                            chunk_seq.append((n, h, r))
            elif _seqv == "2":
                for n in range(NJ):
                    for pair in ((0, 1), (2, 3)):
                        for h in pair:
                            for r in range(CPB * (n + 1) // 2):
                                chunk_seq.append((n, h, r))
            elif _seqv == "3":
                for n in range(NJ):
                    pairs = (((0, 1), (2, 3)) if n % 2 == 0
                             else ((2, 3), (0, 1)))
                    for pair in pairs:
                        for r in range(CPB * (n + 1) // 2):
                            for h in pair:
                                chunk_seq.append((n, h, r))
            else:
                for n in range(NJ):
                    for pair in ((0, 1), (2, 3)):
                        for r in range(CPB * (n + 1) // 2):
                            for h in pair:
                                chunk_seq.append((n, h, r))

            clock["act"] = clock["pe"] + float(_os2.environ.get("KACT0", "0"))
            for (n, h, r) in chunk_seq:
                if n >= NJ // 2:
                    force_qk23(h // 2)
                emit_s_pair(n, h, r)
                chunks_done[(n, h)] += 2
                clock["pe"] += scc(n, r)
                clock["act"] = max(clock["act"], clock["pe"]) + expc(n, r)
                lead = LEAD
                if n == NJ - 1:
                    lead *= float(_os2.environ.get("KLEADF", "1.0"))
                elif n == 0:
                    lead *= float(_os2.environ.get("KLEADF0", "1.0"))
                while clock["pe"] < clock["act"] - lead:
                    c = pop_filler()
                    if c is None:
                        break
                    clock["pe"] += c
            for _ in range(len(fillers) * len(fillers)):
                if pop_filler() is None:
                    break
            assert all(f[3] for f in fillers), "unemitted fillers remain"
    nc.compile()
    return nc


_prog = None


def _get_prog():
    global _prog
    if _prog is None:
        _prog = _build()
    return _prog


def _shard_inputs(x, Wqkv, bqkv, Wproj, bproj):
    xt = [np.ascontiguousarray(x[b].T).astype(_BF16_NP) for b in range(B)]
    tri = np.triu(np.ones((128, 128), np.float32)).astype(_BF16_NP)
    in_maps = []
    for c in range(NCORES):
        b, g = divmod(c, 4)
        dg = slice(DG * g, DG * (g + 1))
        wq_c = np.concatenate(
            [Wqkv[:, dg], Wqkv[:, C:][:, dg], Wqkv[:, 2 * C:][:, dg]], axis=1
        ).astype(_BF16_NP)
        bq, bk = bqkv[dg], bqkv[C:][dg]
        bqk_c = np.stack([bq[:128], bq[128:], bk[:128], bk[128:]], axis=1)
        bqk_c = np.ascontiguousarray(bqk_c, np.float32)
        bv_c = bqkv[2 * C:][dg].astype(_BF16_NP)
        wp_c = np.ascontiguousarray(Wproj[dg, :]).astype(_BF16_NP)
        if g == 0:
            bp_c = np.ascontiguousarray(bproj.reshape(8, 128).T, np.float32)
        else:
            bp_c = np.zeros((128, 8), np.float32)
        in_maps.append({
            "xt": xt[b], "wqkv": wq_c, "bqk": bqk_c, "bv": bv_c,
            "wp": wp_c, "bp": bp_c, "tri": tri,
        })
    return in_maps


def kernel(x, Wqkv, bqkv, Wproj, bproj, _trace=False, _tmpdir=None):
    x = np.asarray(x, np.float32)
    Wqkv = np.asarray(Wqkv, np.float32)
    bqkv = np.asarray(bqkv, np.float32)
    Wproj = np.asarray(Wproj, np.float32)
    bproj = np.asarray(bproj, np.float32)

    nc = _get_prog()
    in_maps = _shard_inputs(x, Wqkv, bqkv, Wproj, bproj)
    res = run_bass_kernel_spmd(
        nc, in_maps, list(range(NCORES)), trace=_trace, tmpdir=_tmpdir,
    )
    outs = [np.asarray(r["out"], np.float32) for r in res.results]
    full = np.empty((B, T, C), np.float32)
    for b in range(B):
        acc = outs[4 * b]
        for g in range(1, 4):
            acc = acc + outs[4 * b + g]
        full[b] = acc.T
    kernel.last_exec_time_ns = res.exec_time_ns
    kernel.last_profile = res.profile_json
    return full



# revision 20
# speedup vs baseline: 1.0393x; 1.0393x over previous
"""GPT2 causal self-attention on 8 TRN2 NeuronCores.

Sharding: tensor-parallel over heads (4 heads/core) x data-parallel over
batch (B=2 -> 2 groups of 4 cores). Each core computes, for its batch
element b and head group g (heads 4g..4g+3):
    qkv slice -> causal attention over its 4 heads -> partial c_proj
and returns the partial [C, T] projection output (transposed, bf16). The
host sums the 4 partials per batch element (the TP all-reduce); bproj is
applied on the g==0 core only.

Design (cost-model engine budget: PE ~89us after fp8-S, Act ~97us):
- QKV: x^T [128,8,T] and Wqkv [128,8,768] arrive in a few large DMAs
  (HWDGE issue is ~625ns serialized, so DMA count matters) split across
  the SP and Activation queues in consumption order.
- Q^T/K^T are written as fp8e4m3 (+bias via the DVE copy) and repacked
  by SBUF->SBUF DMA into a DoubleRow pair layout [32*(h%2)+p, i, t]
  (head-dim d = 32i + p, two heads per tensor at base partitions 0/32),
  so each S^T chunk is a single half-cost DoubleRow fp8 matmul.
- exp runs on Activation: two consecutive same-head S chunks share one
  2-bank psum tile and ONE exp instruction (halves the ~185ns per-exp
  access overhead; chunk b computes its full 512 cols so the shared exp
  never reads uninitialized psum), from a 2-deep pair ring = 4-chunk
  lookahead (depth beats wider tiles: narrow diagonal exps otherwise
  bubble on the S->exp->S feedback). tri-mask on Pool (GPSIMD cannot
  access PSUM on real HW - only sbuf->sbuf work there). y-scale,
  V/yT/proj copies on DVE.
- Emission is driven by a greedy clock model: S/exp chunks paced
  against a modeled Activation clock with a LEAD margin, filler PE work
  (QKV blocks 2-3, V tiles, y groups, transposes, proj groups) popped
  from a dependency-tracked list in the slack slots. Heads are paired,
  chunks head-major within the pair, so transposes/proj unlock early.

Layouts on device (feature-major to avoid transposes):
  xt   = x[b].T            [C=1024, T=2048] bf16 as one [128,8,2048] tile
  Q^T, K^T                 fp8 [128, T] + DoubleRow-paired [128, 2, T]
  V                        token-major [T, 256] as 16x[128, 4, 65] tiles
                           (65th column per head = 1.0 for the softmax denom)
  S^T pair tile            2x[kt=128, qt<=512] psum (2 banks), exp -> P^T bf16
  y accum                  [qt=128, 65] psum (col 64 = denominator)
  y, y^T, out^T            via PE transpose + proj matmul
"""

import numpy as np
import ml_dtypes

import concourse.bass as bass
import concourse.bacc as bacc
import concourse.mybir as mybir
from concourse.tile import TileContext
from concourse.bass_utils import run_bass_kernel_spmd
from concourse.masks import make_identity

B, T, C = 2, 2048, 1024
H, HD = 16, 64
NCORES = 8
HPG = 4            # heads per core
DG = HPG * HD      # 256 feature dims per core
import os as _os
TB = 512           # qkv/proj column block
TBQ = int(_os.environ.get("KTBQ", "512"))   # attention query block width
NJ = T // TBQ      # attention query blocks
CPB = TBQ // 128   # key chunks per query block increment / qt-subs per block
SBUFS = 2   # S pair-tile ring depth (2-bank tiles, 4 banks total)
LEAD = float(_os.environ.get("KLEAD", "3900"))
PWB = int(_os.environ.get("KPW", "44"))
NTT = T // 128     # 16 token tiles

BF16 = mybir.dt.bfloat16
F32 = mybir.dt.float32
FP8 = mybir.dt.float8e4
AF = mybir.ActivationFunctionType

_BF16_NP = ml_dtypes.bfloat16


def _build():
    nc = bacc.Bacc()
    xt_d = nc.declare_dram_parameter("xt", [C, T], BF16, isOutput=False)
    # fp8 DoubleRow-packed copies for the Q/K QKV matmuls: feature
    # d = 256*kc + 128*i + p lives at [p, i, kc, .]; contraction is 256/matmul
    xt8_d = nc.declare_dram_parameter("xt8", [128, 2, 4, T], FP8, isOutput=False)
    wq8_d = nc.declare_dram_parameter("wq8", [128, 2, 4, 4 * 128], FP8, isOutput=False)
    wqlo8_d = nc.declare_dram_parameter("wqlo8", [128, 2, 4, 4 * 128], FP8, isOutput=False)
    wqv_d = nc.declare_dram_parameter("wqv", [C, DG], BF16, isOutput=False)
    bqk_d = nc.declare_dram_parameter("bqk", [128, 4], F32, isOutput=False)
    bv_d = nc.declare_dram_parameter("bv", [DG], BF16, isOutput=False)
    wp_d = nc.declare_dram_parameter("wp", [DG, C], BF16, isOutput=False)
    bp_d = nc.declare_dram_parameter("bp", [128, 8], F32, isOutput=False)
    tri_d = nc.declare_dram_parameter("tri", [128, 128], BF16, isOutput=False)
    out_d = nc.declare_dram_parameter("out", [C, T], BF16, isOutput=True)

    xt_v = xt_d.rearrange("(kc p) t -> p kc t", p=128)     # [128, 8, T]
    wq_v = wqv_d.rearrange("(kc p) c -> p kc c", p=128)    # [128, 8, 256]
    wp_v = wp_d.rearrange("(g p) c -> p g c", p=128)       # [128, 2, 1024]

    with TileContext(nc) as tc:
        with (
            tc.tile_pool(name="const", bufs=1) as cp,
            tc.tile_pool(name="pwork", bufs=PWB) as pw,
            tc.tile_pool(name="ostage", bufs=4) as op,
            tc.tile_pool(name="recips", bufs=8) as rp,
            tc.tile_pool(name="psmm", bufs=2, space="PSUM") as psmm,
            tc.tile_pool(name="pss", bufs=SBUFS, space="PSUM") as pss,
            tc.tile_pool(name="psy", bufs=2, space="PSUM") as psy,
        ):
            # ---- persistent sbuf tensors ----
            xt = cp.tile([128, 8, T], BF16, tag="xt")
            xt8 = cp.tile([128, 2, 4, T], FP8, tag="xt8")
            wq8 = cp.tile([128, 2, 4, 4 * 128], FP8, tag="wq8")
            wqlo8 = cp.tile([128, 2, 4, 4 * 128], FP8, tag="wqlo8")
            wq = cp.tile([128, 8, DG], BF16, tag="wq")
            wp = cp.tile([128, 2, C], BF16, tag="wp")
            q8c = [cp.tile([128, T], FP8, tag=f"q8c{g}", name=f"q8c{g}") for g in range(2)]
            k8c = [cp.tile([128, T], FP8, tag=f"k8c{g}", name=f"k8c{g}") for g in range(2)]
            # head-paired fp8 layout for DoubleRow S matmuls: two heads per
            # tensor at base partitions 0/32 (PE requires base 0/32/64);
            # partition 32*(h%2)+p, pair index i <-> head-dim d = 32i + p
            q8p = [cp.tile([128, 2, T], FP8, tag=f"q8p{a}", name=f"q8p{a}") for a in range(2)]
            k8p = [cp.tile([128, 2, T], FP8, tag=f"k8p{a}", name=f"k8p{a}") for a in range(2)]
            vsb = [cp.tile([128, HPG, HD + 1], BF16, tag=f"v{t}", name=f"v{t}") for t in range(NTT)]
            ysb = [cp.tile([128, DG], BF16, tag=f"y{t}", name=f"y{t}") for t in range(NTT)]
            yT = [cp.tile([128, T], BF16, tag=f"yT{g}", name=f"yT{g}") for g in range(2)]
            tri = cp.tile([128, 128], BF16, tag="tri")
            ident = cp.tile([128, 128], BF16, tag="ident")
            bqk = cp.tile([128, 4], F32, tag="bqk")
            bp = cp.tile([128, 8], F32, tag="bp")
            bv = cp.tile([1, DG], BF16, tag="bv")
            ones = cp.tile([1, 128], BF16, tag="ones")

            # ---- critical input DMAs only: the fp8 QKV operands gate the
            # head (DMA transfers serialize on the shared DMA engines, so
            # bulk bf16 traffic is emitted after the head's shuffle DMAs).
            nc.scalar.dma_start(out=bqk, in_=bqk_d[:, :])
            nc.scalar.dma_start(out=xt8[:, :, :, 0:2 * TB], in_=xt8_d[:, :, :, 0:2 * TB])
            nc.scalar.dma_start(out=tri, in_=tri_d[:, :])
            nc.scalar.dma_start(out=bv, in_=bv_d[:].rearrange("(o n) -> o n", o=1))
            nc.sync.dma_start(out=wq8, in_=wq8_d[:, :, :, :])
            nc.sync.dma_start(out=wqlo8, in_=wqlo8_d[:, :, :, :])
            nc.sync.dma_start(out=xt8[:, :, :, 2 * TB:4 * TB], in_=xt8_d[:, :, :, 2 * TB:4 * TB])

            def bulk_dmas():
                nc.sync.dma_start(out=xt[:, :, 0:TB], in_=xt_v[:, :, 0:TB])
                nc.sync.dma_start(out=wq, in_=wq_v)
                nc.sync.dma_start(out=xt[:, :, TB:2 * TB], in_=xt_v[:, :, TB:2 * TB])
                nc.sync.dma_start(out=xt[:, :, 2 * TB:3 * TB], in_=xt_v[:, :, 2 * TB:3 * TB])
                nc.sync.dma_start(out=xt[:, :, 3 * TB:4 * TB], in_=xt_v[:, :, 3 * TB:4 * TB])
                nc.sync.dma_start(out=wp, in_=wp_v)
                nc.sync.dma_start(out=bp, in_=bp_d[:, :])

            make_identity(nc, ident[:, :])
            nc.vector.memset(ones[:, :], 1.0)
            for t in range(NTT):
                nc.vector.memset(vsb[t][:, :, HD:HD + 1], 1.0)

            def qk_group(n, qk, g):
                # Q^T / K^T feature-major: out[dim, tok] = W.T @ x.T, as 4
                # fp8 DoubleRow matmuls (contraction 256 each, 0.5 cyc/row).
                # Outputs are alpha-scaled (host folds alpha into wq8/bqk;
                # the exp scale divides by alpha^2).
                dst = (q8c if qk == 0 else k8c)[g]
                col = qk * DG + g * 128

                def emitfn():
                    ps = psmm.tile([128, TB], F32, tag="mm", name="ps")
                    for kc in range(4):
                        nc.tensor.matmul(
                            ps,
                            lhsT=wq8[:, :, kc, col:col + 128],
                            rhs=xt8[:, :, kc, n * TB:(n + 1) * TB],
                            start=(kc == 0), stop=False,
                            perf_mode=mybir.MatmulPerfMode.DoubleRow,
                        )
                    # W-residual error feedback: wqlo8 = fp8(aW - fp8(aW))
                    for kc in range(4):
                        nc.tensor.matmul(
                            ps,
                            lhsT=wqlo8[:, :, kc, col:col + 128],
                            rhs=xt8[:, :, kc, n * TB:(n + 1) * TB],
                            start=False, stop=(kc == 3),
                            perf_mode=mybir.MatmulPerfMode.DoubleRow,
                        )
                    nc.vector.tensor_scalar_add(
                        dst[:, n * TB:(n + 1) * TB],
                        ps,
                        bqk[:, 2 * qk + g:2 * qk + g + 1],
                    )

                return [emitfn]

            def v_tile(t):
                # V token-major: out[tok, vdim] = x @ Wv  (+ bv via rank-1)
                ps = psmm.tile([128, DG], F32, tag="mm", name="ps")
                for kc in range(8):
                    nc.tensor.matmul(
                        ps,
                        lhsT=xt[:, kc, t * 128:(t + 1) * 128],
                        rhs=wq[:, kc, 0:DG],
                        start=(kc == 0), stop=False,
                    )
                nc.tensor.matmul(ps, lhsT=ones, rhs=bv, start=False, stop=True)
                nc.vector.tensor_copy(
                    out=vsb[t][:, :, 0:HD],
                    in_=ps.rearrange("p (h d) -> p h d", h=HPG),
                )

            def emit_transpose(t):
                for g in range(2):
                    tp = psy.tile([128, 128], BF16, tag="y", name="tp")
                    nc.tensor.transpose(
                        out=tp, in_=ysb[t][:, g * 128:(g + 1) * 128],
                        identity=ident,
                    )
                    nc.vector.tensor_copy(
                        out=yT[g][:, t * 128:(t + 1) * 128], in_=tp,
                    )

            def proj_group(n, cc, ost=None, c_lo=0, c_hi=TB, dma=True):
                # c_proj: out^T[c, tok] = Wp.T @ y^T for token block n (512),
                # optionally a col sub-range accumulated into a caller tile.
                # The last block alternates psum pools (psy is idle by then)
                # so the ring doesn't serialize the tail.
                ps = psmm.tile([128, c_hi - c_lo], F32, tag="mm", name="ps")
                for g in range(2):
                    nc.tensor.matmul(
                        ps,
                        lhsT=wp[:, g, cc * 128:(cc + 1) * 128],
                        rhs=yT[g][:, n * TB + c_lo:n * TB + c_hi],
                        start=(g == 0), stop=(g == 1),
                    )
                if ost is None:
                    ost = op.tile([128, TB], BF16, tag="o", name="ost")
                if n == T // TB - 1 and cc % 2 == 0:
                    # Act is idle after its last exp; halve the copy tail
                    nc.scalar.activation(
                        out=ost[:, c_lo:c_hi], in_=ps,
                        func=AF.Identity, bias=bp[:, cc:cc + 1],
                    )
                else:
                    nc.vector.tensor_scalar_add(
                        ost[:, c_lo:c_hi], ps, bp[:, cc:cc + 1])
                if dma:
                    eng = nc.scalar if (n == T // TB - 1 and cc % 2 == 0) else nc.sync
                    eng.dma_start(
                        out=out_d[cc * 128:(cc + 1) * 128, n * TB:(n + 1) * TB],
                        in_=ost,
                    )

            pts_live = {(n, h): [] for n in range(NJ) for h in range(HPG)}

            def emit_y(n, h, i, post_i=None):
                # One y accumulation group per qt-sub i: start=True clears
                # has_written for the whole psum bank, so groups must not
                # interleave within a bank.
                pts = pts_live[(n, h)]
                t = CPB * n + i
                last = CPB * n + i
                yps = psy.tile([128, HD + 1], F32, tag="y", name="yps")
                for kk in range(last + 1):
                    off = TBQ * (kk % 2) + i * 128
                    nc.tensor.matmul(
                        yps,
                        lhsT=pts[kk // 2][:, off:off + 128],
                        rhs=vsb[kk][:, h, :],
                        start=(kk == 0), stop=(kk == last),
                    )
                rec = rp.tile([128, 1], F32, tag="r", name="rec")
                nc.vector.reciprocal(out=rec, in_=yps[:, HD:HD + 1])
                nc.vector.tensor_scalar_mul(
                    ysb[t][:, h * HD:(h + 1) * HD],
                    yps[:, 0:HD],
                    rec,
                )
                if post_i is not None:
                    post_i(i, t)

            def emit_s_pair(n, h, r):
                # Two consecutive S^T chunks (keys 256r..256r+256) share one
                # 2-bank psum tile and ONE exp instruction, halving the
                # per-exp access overhead. Chunk b computes its full 512
                # columns (cheap fp8 matmul) so the shared exp never reads
                # uninitialized psum. DoubleRow fp8: contraction d = 32i + p.
                ka, kb = 2 * r, 2 * r + 1
                c0a = max(128 * ka - TBQ * n, 0)
                c0b = max(128 * kb - TBQ * n, 0)
                sp = pss.tile([128, 2 * TBQ], F32, tag="s", name="sp")
                hb = 32 * (h % 2)
                for k, lo in ((ka, c0a), (kb, TBQ)):
                    nc.tensor.matmul(
                        sp[:, lo:(TBQ if k == ka else 2 * TBQ)],
                        lhsT=k8p[h // 2][hb:hb + 32, :, k * 128:(k + 1) * 128],
                        rhs=q8p[h // 2][hb:hb + 32, :,
                                        TBQ * n + (lo - (0 if k == ka else TBQ)):TBQ * (n + 1)],
                        start=True, stop=True,
                        perf_mode=mybir.MatmulPerfMode.DoubleRow,
                    )
                pt = pw.tile([128, 2 * TBQ], BF16, tag="p", name=f"pt{r}")
                nc.scalar.activation(
                    out=pt[:, c0a:2 * TBQ], in_=sp[:, c0a:2 * TBQ],
                    func=AF.Exp, scale=0.125 / (ALPHA * ALPHA),
                )
                for k, base, c0 in ((ka, 0, c0a), (kb, TBQ, c0b)):
                    if 128 * k >= TBQ * n:
                        nc.gpsimd.tensor_mul(
                            out=pt[:, base + c0:base + c0 + 128],
                            in0=pt[:, base + c0:base + c0 + 128],
                            in1=tri,
                        )
                pts_live[(n, h)].append(pt)

            # ---- greedy interleaved emission ----
            # Pace S/exp chunk emission against a clock model of the
            # Activation engine: after each chunk, emit filler PE work until
            # the modeled PE clock catches the Act clock. Heads are paired
            # and chunk emission alternates within a pair (k-major) so yT
            # transposes and proj unlock early.
            import os
            expf = float(os.environ.get("KEXPF", "1.05"))

            def expc(n, r):
                c0a = max(256 * r - TBQ * n, 0)
                return ((2 * TBQ - c0a) * 0.833 + 185) * expf

            def scc(n, r):
                c0a = max(256 * r - TBQ * n, 0)
                return (2 * TBQ - c0a) * 0.104 + 120

            chunks_done = {(n, h): 0 for n in range(NJ) for h in range(HPG)}
            y_done = {(n, h): 0 for n in range(NJ) for h in range(HPG)}
            state = {"v": 0, "T": 0}

            fillers = []        # (cost_ns, ready_fn, emit_fn, emitted)

            def add_filler(cost, ready, emit):
                fillers.append([cost, ready, emit, False])

            # qkv column blocks 2,3 (needed before any n>=2 chunk)
            qk23 = {0: [], 1: []}
            for g in range(2):
                for qk in range(2):
                    for nn in (2, 3):
                        qk23[g].append(qk_group(nn, qk, g))
            shuffled1 = {0: False, 1: False}

            def shuf_g(g, lo, hi):
                # repack q8c/k8c into the DoubleRow pair layout with the
                # bijection d = 2*pi + i: the [64, cols] -> [32, 2, cols]
                # element orders then coincide, so ONE DMA per (head, tensor)
                # does the whole permutation.
                for hh in (2 * g, 2 * g + 1):
                    po = (hh % 2) * 64
                    hb = 32 * (hh % 2)
                    for src_, dst in ((q8c, q8p), (k8c, k8p)):
                        nc.sync.dma_start(
                            out=dst[g][hb:hb + 32, :, lo:hi],
                            in_=src_[g][po:po + 64, lo:hi],
                        )

            def emit_shuffle1_g(g):
                shuf_g(g, T // 2, T)

            def force_qk23(g):
                if shuffled1[g]:
                    return
                for f in fillers:
                    if not f[3] and getattr(f[2], "__name__", "") == f"qk23_{g}":
                        f[2]()
                        f[3] = True
                emit_shuffle1_g(g)
                shuffled1[g] = True

            qk23_left = {0: 4, 1: 4}
            for g in range(2):
                for pieces in qk23[g]:
                    def mk(pieces=pieces, g=g):
                        def emitfn():
                            for p in pieces:
                                p()
                            qk23_left[g] -= 1
                            if qk23_left[g] == 0 and not shuffled1[g]:
                                emit_shuffle1_g(g)
                                shuffled1[g] = True
                        emitfn.__name__ = f"qk23_{g}"
                        return emitfn
                    add_filler(940, lambda: True, mk())

            # modeled arrival times of the bulk bf16 DMAs (serialized DMA
            # engine), gating fillers so in-order PE never stalls on them
            arr0 = float(_os.environ.get("KARR0", "14500"))
            arrd = float(_os.environ.get("KARRD", "2900"))
            arr_xt = [arr0 + arrd * q for q in range(4)]
            arr_wp = arr0 + 4 * arrd + 1500

            for t in range(NTT):
                def mkv(t=t):
                    def emitfn():
                        v_tile(t)
                        state["v"] += 1
                    return emitfn
                add_filler(
                    960,
                    lambda t=t: clock["pe"] >= arr_xt[t // 4],
                    mkv(),
                )

            for n in range(NJ):
                for i in range(CPB):
                    t = CPB * n + i
                    for h in range(HPG):
                        def mky(n=n, h=h, i=i):
                            def emitfn():
                                emit_y(n, h, i)
                                y_done[(n, h)] += 1
                            return emitfn
                        add_filler(
                            27 * (CPB * n + i + 1) + 120,
                            lambda n=n, h=h, i=i: (
                                chunks_done[(n, h)] > CPB * n + i
                                and state["v"] > CPB * n + i
                            ),
                            mky(),
                        )
                    def mkt(t=t, n=n, i=i):
                        def emitfn():
                            emit_transpose(t)
                            state["T"] += 1
                        return emitfn
                    add_filler(
                        160,
                        lambda n=n, i=i: all(
                            y_done[(n, h)] > i for h in range(HPG)
                        ),
                        mkt(),
                    )
                    if t % 4 == 3:
                        for cc in range(8):
                            def mkp(nb=t // 4, cc=cc):
                                def emitfn():
                                    proj_group(nb, cc)
                                return emitfn
                            add_filler(
                                426,
                                lambda t=t: (state["T"] >= t + 1
                                             and clock["pe"] >= arr_wp),
                                mkp(),
                            )

            def pop_filler():
                for f in fillers:
                    if not f[3] and f[1]():
                        f[2]()
                        f[3] = True
                        return f[0]
                return None

            # head: g-major QKV for the first two column blocks, then the
            # j<2 shuffle, so the first head pair's chunks unlock after 4
            # matmul groups; bulk bf16 DMAs go behind the shuffles
            clock = {"pe": 0.0, "act": 0.0}
            for g in range(2):
                for qk in range(2):
                    for nn in (0, 1):
                        for piece in qk_group(nn, qk, g):
                            piece()
                shuf_g(g, 0, T // 2)
                clock["pe"] += 4 * 1040
            bulk_dmas()

            import os as _os2
            _seqv = _os2.environ.get("KSEQ", "0")
            chunk_seq = []           # units are chunk PAIRS (r = k//2)
            if _seqv == "1":
                for n in range(NJ):
                    for r in range(CPB * (n + 1) // 2):
                        for h in range(HPG):
                            chunk_seq.append((n, h, r))
            elif _seqv == "2":
                for n in range(NJ):
                    for pair in ((0, 1), (2, 3)):
                        for h in pair:
                            for r in range(CPB * (n + 1) // 2):
                                chunk_seq.append((n, h, r))
            elif _seqv == "3":
                for n in range(NJ):
                    pairs = (((0, 1), (2, 3)) if n % 2 == 0
                             else ((2, 3), (0, 1)))
                    for pair in pairs:
                        for r in range(CPB * (n + 1) // 2):
                            for h in pair:
                                chunk_seq.append((n, h, r))
            else:
                for n in range(NJ):
                    for pair in ((0, 1), (2, 3)):
                        for r in range(CPB * (n + 1) // 2):
                            for h in pair:
                                chunk_seq.append((n, h, r))

            clock["act"] = clock["pe"] + float(_os2.environ.get("KACT0", "0"))
            for (n, h, r) in chunk_seq:
                if n >= NJ // 2:
                    force_qk23(h // 2)
                emit_s_pair(n, h, r)
                chunks_done[(n, h)] += 2
                clock["pe"] += scc(n, r)
                clock["act"] = max(clock["act"], clock["pe"]) + expc(n, r)
                lead = LEAD
                if n == NJ - 1:
                    lead *= float(_os2.environ.get("KLEADF", "1.0"))
                elif n == 0:
                    lead *= float(_os2.environ.get("KLEADF0", "1.0"))
                while clock["pe"] < clock["act"] - lead:
                    c = pop_filler()
                    if c is None:
                        break
                    clock["pe"] += c
            for _ in range(len(fillers) * len(fillers)):
                if pop_filler() is None:
                    break
            assert all(f[3] for f in fillers), "unemitted fillers remain"
    nc.compile()
    return nc


_prog = None


def _get_prog():
    global _prog
    if _prog is None:
        _prog = _build()
    return _prog


ALPHA = 64.0
_FP8_NP = ml_dtypes.float8_e4m3


def _dr_pack(m):
    # [1024, ncols] -> [128, 2, 4, ncols]: feature d = 256*kc + 128*i + p
    ncols = m.shape[1]
    return np.ascontiguousarray(
        m.reshape(4, 2, 128, ncols).transpose(2, 1, 0, 3)
    )


def _shard_inputs(x, Wqkv, bqkv, Wproj, bproj):
    xt = [np.ascontiguousarray(x[b].T).astype(_BF16_NP) for b in range(B)]
    xt8 = [_dr_pack(x[b].T.astype(_FP8_NP).astype(np.float32)).astype(_FP8_NP)
           for b in range(B)]
    tri = np.triu(np.ones((128, 128), np.float32)).astype(_BF16_NP)
    in_maps = []
    for c in range(NCORES):
        b, g = divmod(c, 4)
        dg = slice(DG * g, DG * (g + 1))
        wqk_a = np.concatenate([Wqkv[:, dg], Wqkv[:, C:][:, dg]], axis=1) * ALPHA
        wqk_hi = wqk_a.astype(_FP8_NP)
        wq8_c = _dr_pack(wqk_hi.astype(np.float32)).astype(_FP8_NP)
        wqlo8_c = _dr_pack(wqk_a - wqk_hi.astype(np.float32)).astype(_FP8_NP)
        wqv_c = np.ascontiguousarray(Wqkv[:, 2 * C:][:, dg]).astype(_BF16_NP)
        bq, bk = bqkv[dg] * ALPHA, bqkv[C:][dg] * ALPHA
        bqk_c = np.stack([bq[:128], bq[128:], bk[:128], bk[128:]], axis=1)
        bqk_c = np.ascontiguousarray(bqk_c, np.float32)
        bv_c = bqkv[2 * C:][dg].astype(_BF16_NP)
        wp_c = np.ascontiguousarray(Wproj[dg, :]).astype(_BF16_NP)
        if g == 0:
            bp_c = np.ascontiguousarray(bproj.reshape(8, 128).T, np.float32)
        else:
            bp_c = np.zeros((128, 8), np.float32)
        in_maps.append({
            "xt": xt[b], "xt8": xt8[b], "wq8": wq8_c, "wqlo8": wqlo8_c,
            "wqv": wqv_c,
            "bqk": bqk_c, "bv": bv_c,
            "wp": wp_c, "bp": bp_c, "tri": tri,
        })
    return in_maps


def kernel(x, Wqkv, bqkv, Wproj, bproj, _trace=False, _tmpdir=None):
    x = np.asarray(x, np.float32)
    Wqkv = np.asarray(Wqkv, np.float32)
    bqkv = np.asarray(bqkv, np.float32)
    Wproj = np.asarray(Wproj, np.float32)
    bproj = np.asarray(bproj, np.float32)

    nc = _get_prog()
    in_maps = _shard_inputs(x, Wqkv, bqkv, Wproj, bproj)
    res = run_bass_kernel_spmd(
        nc, in_maps, list(range(NCORES)), trace=_trace, tmpdir=_tmpdir,
    )
    outs = [np.asarray(r["out"], np.float32) for r in res.results]
    full = np.empty((B, T, C), np.float32)
    for b in range(B):
        acc = outs[4 * b]
        for g in range(1, 4):
            acc = acc + outs[4 * b + g]
        full[b] = acc.T
    kernel.last_exec_time_ns = res.exec_time_ns
    kernel.last_profile = res.profile_json
    return full

